# revision 45
# baseline (speedup 1.0000x reference)
"""Causal dot-product attention (Keras Luong Attention, key=value, causal=True)
on 8 Trainium2 NeuronCores, data-parallel over batch (B=8 -> 1 batch/core).

Per core: query [T, D] f32, value [T, D] f32 -> out [T, D] f32
  S = Q @ V^T (causal), P = softmax(S), out = P @ V      (T=2048, D=1024)

Precision: S matmul in float32r (fp32 storage, reduced-mantissa PE mode, full
speed at N>=256), PV matmul in bf16. End-to-end rel_l2 ~1.8e-3 vs f32 reference.

Single-core dataflow, software-pipelined per 128-row q-tile i:
  - input tiles stream in with lead distance (V: 4 tiles, Q: 2 tiles); each is
    PE-transposed (8 [128,128] transposes -> PSUM -> DVE/ACT copy to f32r)
  - S[:, 0:W] = Q_i @ V^T, f32r matmuls into 4 PSUM banks, W=(i+1)*128; the
    causal -1e9 mask is accumulated into the diagonal chunk as I.T @ maskneg
    (keeps the mask off the DVE softmax chain)
  - row max (split in halves, DVE) -> exp(S - max) on ACT in halves with fused
    row-sum accum -> reciprocal; P written as bf16
  - P^T via one bf16 DMA-transpose on the SP HWDGE ring (out-stores ride SWDGE
    so their waits can't head-of-line block the PT transposes)
  - out_i = (P @ V) * rcp with bf16 matmuls, deferred two iterations (PV_LAG)
    so the softmax+transpose latency hides behind later S matmuls

reps>1 wraps the body in a hardware For_i loop (benchmark-only path).
Measured: ~210 us/core on HW (rep-delta method), rel err 1.8e-3.
"""
import numpy as np

B, T, D = 8, 2048, 1024
N_CORES = 8
NEG = 1.0e9

_NC_CACHE = {}


def _build_attention(T=T, D=D, reps=1, ablate="full", S_C_OUTER=False):
    import contextlib
    import concourse.bacc as bacc
    import concourse.tile as tile
    import concourse.mybir as mybir

    F32 = mybir.dt.float32
    F32R = mybir.dt.float32r
    BF16 = mybir.dt.bfloat16

    nc = bacc.Bacc(debug=False)
    NT = T // 128      # number of 128-row seq tiles
    ND = D // 128      # number of 128-wide d chunks
    ND2 = D // 512     # number of 512-wide output chunks

    q_dram = nc.dram_tensor("query", [T, D], F32, kind="ExternalInput")
    v_dram = nc.dram_tensor("value", [T, D], F32, kind="ExternalInput")
    o_dram = nc.dram_tensor("out", [T, D], F32, kind="ExternalOutput")

    ident_np = np.eye(128, dtype=np.float32)
    maskneg_np = np.where(
        np.arange(128)[:, None] >= np.arange(128)[None, :], 0.0, -NEG
    ).astype(np.float32)
    ident_dram = nc.inline_tensor(ident_np, name="ident")
    maskneg_dram = nc.inline_tensor(maskneg_np, name="maskneg")

    with tile.TileContext(nc) as tc:
        with (
            tc.tile_pool(name="const", bufs=1) as constp,
            tc.tile_pool(name="scr", bufs=6) as scrp,
            tc.tile_pool(name="big", bufs=1) as bigp,
            tc.tile_pool(name="qt", bufs=4) as qtp,
            tc.tile_pool(name="pbf", bufs=4) as pbfp,
            tc.tile_pool(name="pt", bufs=5) as ptp,
            tc.tile_pool(name="osb", bufs=2) as osbp,
            tc.tile_pool(name="stat", bufs=6) as statp,
            tc.tile_pool(name="tp_ps", bufs=2, space="PSUM") as tpp,
            tc.tile_pool(name="s_ps", bufs=1, space="PSUM") as spp,
            tc.tile_pool(name="pv_ps", bufs=1, space="PSUM") as pvp,
        ):
            maskneg = constp.tile([128, 128], F32)
            nc.sync.dma_start(maskneg[:], maskneg_dram[:])
            ident = constp.tile([128, 128], F32)
            nc.sync.dma_start(ident[:], ident_dram[:])
            # f32r copies for the mask-as-matmul trick
            ident_r = constp.tile([128, 128], F32R)
            maskneg_r = constp.tile([128, 128], F32R)
            nc.vector.tensor_copy(ident_r[:], ident[:])
            nc.vector.tensor_copy(maskneg_r[:], maskneg[:])

            vt = bigp.tile([128, ND, T], F32R)
            vbf = bigp.tile([128, NT, D], BF16)

            rep_ctx = tc.For_i(0, reps, 1) if reps > 1 else contextlib.nullcontext()

            def transpose_tile(src_f32, dst, dst_qslice):
                # src [128, D] f32 -> dst[:, :, dst_qslice] f32r via PE transpose
                for g in range(ND // 4):
                    tp = tpp.tile([128, 512], F32, tag="tp")
                    for c in range(4):
                        cc = 4 * g + c
                        nc.tensor.transpose(
                            tp[:, c * 128:(c + 1) * 128],
                            src_f32[:, cc * 128:(cc + 1) * 128],
                            ident[:],
                        )
                    if g % 2 == 0:
                        nc.vector.tensor_copy(dst[:, 4 * g:4 * g + 4, dst_qslice], tp[:])
                    else:
                        nc.scalar.copy(dst[:, 4 * g:4 * g + 4, dst_qslice], tp[:])

            def emit_pv_epilogue(pend):
                pt, rcp, i = pend
                opv = pvp.tile([128, D], F32, tag="opv")
                for j in range(i + 1):
                    for n in range(ND2):
                        nc.tensor.matmul(
                            opv[:, n * 512:(n + 1) * 512],
                            pt[:, j, :],
                            vbf[:, j, n * 512:(n + 1) * 512],
                            start=(j == 0),
                            stop=(j == i),
                        )
                o_sb = osbp.tile([128, D], F32, tag="osb")
                nc.vector.tensor_scalar_mul(o_sb[:], opv[:], rcp[:])
                # store via SWDGE (Pool) so its wait on the scale can't
                # head-of-line block the SP ring that carries the PT DMAs
                nc.gpsimd.dma_start(o_dram[i * 128:(i + 1) * 128, :], o_sb[:])

            with rep_ctx:
                V_LEAD, Q_LEAD = 4, 2
                qts = []

                def prep_v(j):
                    vs = scrp.tile([128, D], F32, tag="scr")
                    nc.sync.dma_start(vs[:], v_dram[j * 128:(j + 1) * 128, :])
                    nc.vector.tensor_copy(vbf[:, j, :], vs[:])
                    transpose_tile(vs, vt, slice(j * 128, (j + 1) * 128))

                def prep_q(j):
                    qs = scrp.tile([128, D], F32, tag="scr")
                    nc.sync.dma_start(qs[:], q_dram[j * 128:(j + 1) * 128, :])
                    qt_n = qtp.tile([128, ND, 128], F32R, tag="qt")
                    transpose_tile(qs, qt_n, slice(0, 128))
                    qts.append(qt_n)

                # ------- prologue: lead tiles only (rest is pipelined) -------
                if ablate == "mm_only":
                    zf = scrp.tile([128, D], F32, tag="scr")
                    nc.vector.memset(zf[:], 0.0)
                    zf3 = zf[:, :].rearrange("p (c q) -> p c q", q=128)
                    nc.vector.memset(vbf[:], 0.0)
                    for c in range(ND):
                        for h in range(T // D):
                            nc.vector.tensor_copy(
                                vt[:, c, h * D:(h + 1) * D], zf[:, :]
                            )
                    for j in range(NT):
                        qt_n = qtp.tile([128, ND, 128], F32R, tag="qt")
                        nc.vector.tensor_copy(qt_n[:], zf3)
                        qts.append(qt_n)
                else:
                    prep_v(0)
                    prep_q(0)
                    for j in range(1, V_LEAD):
                        prep_v(j)
                    for j in range(1, Q_LEAD):
                        prep_q(j)

                # ---------------- main loop ----------------
                pending = []  # [(pt, rcp, i)] awaiting PV+epilogue (lag 2)
                PV_LAG = 2
                for i in range(NT):
                    W = (i + 1) * 128

                    # S = Q_i @ V^T over columns [0, W); the causal -1e9 mask is
                    # accumulated into the diagonal chunk as I.T @ maskneg
                    s_ps = spp.tile([128, T], F32, tag="s")
                    qt_i = qts[i]
                    nchunk = (W + 511) // 512
                    if S_C_OUTER:
                        for c in range(ND):
                            for ch in range(nchunk):
                                n0 = ch * 512
                                n1 = min(W, n0 + 512)
                                nc.tensor.matmul(
                                    s_ps[:, n0:n1],
                                    qt_i[:, c, :],
                                    vt[:, c, n0:n1],
                                    start=(c == 0),
                                    stop=False,
                                    skip_group_check=True,
                                )
                        nc.tensor.matmul(
                            s_ps[:, i * 128:W], ident_r[:], maskneg_r[:],
                            start=False, stop=True, skip_group_check=True,
                        )
                    else:
                        for ch in range(nchunk):
                            n0 = ch * 512
                            n1 = min(W, n0 + 512)
                            last_ch = ch == nchunk - 1
                            for c in range(ND):
                                nc.tensor.matmul(
                                    s_ps[:, n0:n1],
                                    qt_i[:, c, :],
                                    vt[:, c, n0:n1],
                                    start=(c == 0),
                                    stop=(c == ND - 1) and not last_ch,
                                )
                            if last_ch:
                                nc.tensor.matmul(
                                    s_ps[:, i * 128:W], ident_r[:], maskneg_r[:],
                                    start=False, stop=True,
                                )

                    # prep upcoming Q and V tiles (PE transposes interleave
                    # with this iteration's softmax on DVE/ACT)
                    if ablate != "mm_only":
                        if i + Q_LEAD < NT:
                            prep_q(i + Q_LEAD)
                        if i + V_LEAD < NT:
                            prep_v(i + V_LEAD)

                    if ablate in ("s_only", "mm_only"):
                        continue
                    # exact row max, split in halves: the first half's max runs
                    # while the PE still computes the later S chunks
                    negmax = statp.tile([128, 1], F32, tag="negmax")
                    nhalf = 1 if W <= 1024 else 2
                    if nhalf == 1:
                        nc.vector.tensor_reduce(
                            negmax[:], s_ps[:, 0:W], axis=mybir.AxisListType.X,
                            op=mybir.AluOpType.max, negate=True,
                        )
                    else:
                        pmax = statp.tile([128, 2], F32, tag="pmax")
                        nc.vector.tensor_reduce(
                            pmax[:, 0:1], s_ps[:, 0:1024],
                            axis=mybir.AxisListType.X, op=mybir.AluOpType.max,
                        )
                        nc.vector.tensor_reduce(
                            pmax[:, 1:2], s_ps[:, 1024:W],
                            axis=mybir.AxisListType.X, op=mybir.AluOpType.max,
                        )
                        nc.vector.tensor_reduce(
                            negmax[:], pmax[:, 0:2], axis=mybir.AxisListType.X,
                            op=mybir.AluOpType.max, negate=True,
                        )
                    p_bf = pbfp.tile([128, T], BF16, tag="pbf")
                    psums = statp.tile([128, 2], F32, tag="psums")
                    for h in range(nhalf):
                        h0 = h * 1024
                        h1 = min(W, h0 + 1024)
                        nc.scalar.activation(
                            p_bf[:, h0:h1], s_ps[:, h0:h1],
                            mybir.ActivationFunctionType.Exp,
                            bias=negmax[:], scale=1.0,
                            accum_out=psums[:, h:h + 1],
                        )
                    sumexp = statp.tile([128, 1], F32, tag="sumexp")
                    if nhalf == 2:
                        nc.vector.tensor_reduce(
                            sumexp[:], psums[:, 0:2], axis=mybir.AxisListType.X,
                            op=mybir.AluOpType.add,
                        )
                    else:
                        sumexp = psums[:, 0:1]
                    rcp = statp.tile([128, 1], F32, tag="rcp")
                    nc.vector.reciprocal(rcp[:], sumexp[:])

                    if ablate == "s_soft":
                        continue
                    # P^T via DMA transpose (bf16)
                    pt = ptp.tile([128, NT, 128], BF16, tag="pt")
                    nc.sync.dma_start(pt[:, 0:i + 1, :], p_bf[:, 0:W], transpose=True)

                    # emit PV lagged so softmax(i) hides behind later S matmuls
                    pending.append((pt, rcp, i))
                    if len(pending) > PV_LAG:
                        emit_pv_epilogue(pending.pop(0))

                for pend in pending:
                    emit_pv_epilogue(pend)
                if ablate != "full":
                    # keep outputs defined so the NEFF writes something
                    o_sb = osbp.tile([128, D], F32, tag="osb")
                    nc.vector.tensor_copy(o_sb[:], vbf[:, 0, :])
                    nc.gpsimd.dma_start(o_dram[0:128, :], o_sb[:])
    nc.finalize()
    return nc


def _get_nc():
    if "nc" not in _NC_CACHE:
        _NC_CACHE["nc"] = _build_attention()
    return _NC_CACHE["nc"]


def kernel(query, value):
    from concourse.bass_utils import run_bass_kernel_spmd

    query = np.asarray(query, dtype=np.float32)
    value = np.asarray(value, dtype=np.float32)
    assert query.shape == (B, T, D) and value.shape == (B, T, D)

    nc = _get_nc()
    in_maps = [
        {"query": np.ascontiguousarray(query[i]),
         "value": np.ascontiguousarray(value[i])}
        for i in range(N_CORES)
    ]
    res = run_bass_kernel_spmd(nc, in_maps, core_ids=list(range(N_CORES)))
    return np.stack([res.results[i]["out"] for i in range(N_CORES)]).astype(np.float32)
